# revision 24
# baseline (speedup 1.0000x reference)
"""AxonLIFNode forward on 8 Trainium2 NeuronCores.

Reference recurrence (per element, sequential over T):
    mem   = mem + (x_t + V_RESET - mem) / TAU        # V_RESET=0, TAU=2
    spike = (mem - V_TH > 0)                         # V_TH=1, {0.0, 1.0}
    mem   = (1 - spike) * mem + V_RESET * spike      # reset to 0 on spike
    out_i = out_i * sigmoid(w) + spike               # axon current (w=0 -> 0.5)
    outputs: (spike, out_i), both [B, T, N] f32

Strategy: data-parallel over the batch axis (B=64 -> 8 per core). Per core the
32768 independent series are laid out as 128 partitions x 256 free elements.
Device compute is DVE-only, two custom ops:

  LIF_M1: m1_t = prev + (x_t - prev) * 0.5, prev = m1_{t-1} * (m1_{t-1} <= 1)
  LIF_OI: oi_t = oi_{t-1} * inv_tau + (m1_t > 1)

Both recurrences run GROUPED: one instruction covers G timesteps by writing
m1_buf slots [t0+1, t0+G] while reading slots [t0, t0+G-1] -- the later
timesteps of the input stream read values the same instruction wrote F=256
elements (cycles) earlier, far beyond the DVE pipeline depth. This amortizes
the ~133 ns fixed cost per DVE instruction over 8 timesteps (vs 1 in the
per-step formulation), cutting DVE busy from ~46 us to ~37 us.

The spike tensor is never written to HBM: oi is stored as f16 (lossless
enough: fp32 internal math, f16 state rounding, measured e_oi ~9e-5), and the
host recovers spikes exactly via spike_t = rint(oi_t - inv_tau * oi_{t-1})
(max recovery residual ~5e-4 << 0.5). That cuts HBM traffic per core from
18 MB (8 in + 2 spk + 8 oi) to 12 MB (8 in + 4 oi f16).
"""

import numpy as np

import concourse.bacc as bacc
import concourse.mybir as mybir
import concourse.dve_ops as dve_ops
from concourse.dve_ops import DveOp
from concourse.dve_spec import Spec, Src0, Src1, C0, C1, lower
from concourse.dve_uop import DveOpSpec
from concourse.tile import TileContext
from concourse.bass_utils import run_bass_kernel_spmd

# Problem shape (hardcoded per harness contract).
B, T, N = 64, 64, 4096
CORES = 8
BS = B // CORES          # batches per core
P = 128                  # SBUF partitions
J = 16                   # n-chunks per batch: BS * J == P
F = N // J               # free elements per partition per timestep (256)
# M1 group sizes: small leading groups so compute starts after ~128 KB of X
# has landed instead of a full 1 MB group; small trailing groups so the last
# output chunks (stored in parallel on both rings) are small.
M1_TIERS = [2, 2, 4, 4, 4, 8, 8, 8, 8, 8, 4, 4]
OI_TIERS = [8, 8, 16, 16, 8, 4, 4]
# Input DMA batches: (t_lo, t_hi, ring). Each HWDGE ring is a FIFO whose
# transfers cost roughly (bytes/share-BW + ~1.6 us completion receipt), and
# ALL in-flight transfers share the 16 SDMA engines at packet granularity
# regardless of ring. So the latency-critical ramp (t < 16) is small
# alternating-ring chunks with NO bulk transfer competing, and the 1 MB bulk
# chunks are queued BEHIND the ramp on the same two rings, each arriving
# comfortably before the (slower) compute chain reaches its slot.
IN_BATCHES = [
    (0, 2, "sync"),
    (2, 4, "scalar"),
    (4, 8, "sync"),
    (8, 12, "scalar"),
    (12, 16, "sync"),
    (16, 24, "scalar"),
    (24, 32, "sync"),
    (32, 40, "scalar"),
    (40, 48, "sync"),
    (48, 56, "scalar"),
    (56, 64, "sync"),
]
# Output stores: (t_end_trigger, t_lo, t_hi, ring): 1 MB chunks mid-stream,
# small chunks at the tail; the final chunk is split by partition halves
# across both rings so the drain parallelizes.
OI_STORES = [
    (16, 0, 16, "scalar"),
    (32, 16, 32, "sync"),
    (48, 32, 48, "scalar"),
    (56, 48, 56, "sync"),
    (60, 56, 60, "scalar"),
    (64, 60, 64, "both"),
]
assert sum(M1_TIERS) == T and sum(OI_TIERS) == T


def _register_op(name: str, spec: Spec) -> DveOp:
    """Register a custom DVE op in the global registry with a computed sha."""
    for op in dve_ops.OPS:
        if op.name == name:
            return op
    row = dve_ops._CUSTOM_DVE_ROW_BASE + len(dve_ops.OPS)
    assert row < 0x20, "custom-DVE opcode rows exhausted"
    shas = {}
    for ver in ("v3", "v4"):
        uops = lower(spec, ver=ver)
        shas[ver] = DveOpSpec(name=name, opcode=row, uops=uops, rd1_en=True).sha(ver)
    op = DveOp(name, spec, subdim=False, uops_sha=shas)
    dve_ops._SUB_OPCODE_FOR_NAME[name] = row
    dve_ops.OPS.append(op)
    dve_ops.CUSTOM_DVE_SPECS[name] = spec
    return op


def _lif_ops() -> tuple[DveOp, DveOp]:
    """LIF_M1: m1_t from (x_t, m1_{t-1}); LIF_OI: oi_t from (oi_{t-1}, m1_t).

    LIF_M1: out = prev + (Src0 - prev) * C0, prev = Src1 * (Src1 <= C1)
    LIF_OI: out = Src0 * C0 + (Src1 > C1)
    Each ALU stage is one IEEE f32 rounding; bit-exact vs the reference.
    """
    keep = Src1 <= C1
    prev = Src1 * keep
    m1 = _register_op(
        "LIF_M1_ANT",
        Spec(
            body=prev + (Src0 - prev) * C0,
            reference=lambda in0, in1, s0, s1, imm2: (
                (p := (in1 * (in1 <= s1)).astype(np.float32))
                + (in0 - p) * np.float32(s0)
            ).astype(np.float32),
        ),
    )
    oi = _register_op(
        "LIF_OI_ANT",
        Spec(
            body=Src0 * C0 + (Src1 > C1),
            reference=lambda in0, in1, s0, s1, imm2: (
                in0 * np.float32(s0) + (in1 > s1)
            ).astype(np.float32),
        ),
    )
    return m1, oi


_nc_cache: dict = {}


def _build(inv_tau: float):
    """Trace + compile the per-core Bass program (SPMD: same NEFF, 8 cores)."""
    key = float(inv_tau)
    if key in _nc_cache:
        return _nc_cache[key]

    lif_m1, lif_oi = _lif_ops()
    f32 = mybir.dt.float32
    f16 = mybir.dt.float16

    nc = bacc.Bacc(
        "TRN2",
        target_bir_lowering=False,
        debug=False,
        enable_asserts=False,
        num_devices=CORES,
    )
    # Host pre-transposes each core's shard to [(b j) = 128, T, F] contiguous,
    # so every DMA is a 3-dim AP with an 8 KB contiguous run per partition.
    x_r = nc.dram_tensor("x", [P, T, F], f32, kind="ExternalInput").ap()
    # oi stored as f16; spikes are recovered from it on the host.
    oi_r = nc.dram_tensor("oi", [P, T, F], f16, kind="ExternalOutput").ap()

    with TileContext(nc) as tc:
        with (
            tc.tile_pool(name="xin", bufs=1) as xpool,
            tc.tile_pool(name="state", bufs=1) as spool,
        ):
            # The whole per-core X fits in SBUF (64 KB/partition). Issue every
            # input DMA up front across the two HWDGE rings.
            engines = {"sync": nc.sync, "scalar": nc.scalar, "gpsimd": nc.gpsimd}
            x_tiles = []  # (tile, t_start, t_len)
            for ts, te, ring in IN_BATCHES:
                xt = xpool.tile([P, te - ts, F], f32, name=f"x_{ts}", bufs=1)
                x_tiles.append((xt, ts, te - ts))
                engines[ring].dma_start(out=xt[:], in_=x_r[:, ts:te, :])

            def x_slice(t0, tlen):
                for xt, ts, tl in x_tiles:
                    if ts <= t0 and t0 + tlen <= ts + tl:
                        return xt[:, t0 - ts : t0 - ts + tlen, :]
                raise AssertionError((t0, tlen))

            # Linear state buffers; slot 0 = zero initial state, step t lives
            # at slot t+1. Both recurrences self-reference within grouped
            # instructions (read slot t while writing slot t+1, F elements
            # apart in the stream).
            m1_buf = spool.tile([P, T + 1, F], f32)
            oi_buf = spool.tile([P, T + 1, F], f16)
            nc.vector.memset(m1_buf[:, 0, :], 0.0)
            nc.vector.memset(oi_buf[:, 0, :], 0.0)

            oi_iter = iter(OI_TIERS)
            oi_done = 0
            oi_next = next(oi_iter)
            store_iter = iter(OI_STORES)
            store_next = next(store_iter)

            def issue_oi_until(t_max):
                nonlocal oi_done, oi_next, store_next
                while oi_next is not None and oi_done + oi_next <= t_max:
                    t0, gl = oi_done, oi_next
                    # DVE: oi_t = oi_{t-1} * inv_tau + (m1_t > 1); f16 state,
                    # fp32 internal math, one instruction per group.
                    nc.vector._custom_dve(
                        lif_oi,
                        out=oi_buf[:, t0 + 1 : t0 + 1 + gl, :],
                        in0=oi_buf[:, t0 : t0 + gl, :],
                        in1=m1_buf[:, t0 + 1 : t0 + 1 + gl, :],
                        s0=inv_tau,
                        s1=1.0,
                    )
                    oi_done += gl
                    oi_next = next(oi_iter, None)
                    while store_next is not None and store_next[0] <= oi_done:
                        _, lo, hi, ring = store_next
                        if ring == "both":
                            h = P // 2
                            nc.sync.dma_start(
                                out=oi_r[:h, lo:hi, :],
                                in_=oi_buf[:h, lo + 1 : hi + 1, :],
                            )
                            nc.scalar.dma_start(
                                out=oi_r[h:, lo:hi, :],
                                in_=oi_buf[h:, lo + 1 : hi + 1, :],
                            )
                        else:
                            engines[ring].dma_start(
                                out=oi_r[:, lo:hi, :],
                                in_=oi_buf[:, lo + 1 : hi + 1, :],
                            )
                        store_next = next(store_iter, None)

            t0 = 0
            for gl in M1_TIERS:
                # DVE: m1_t = prev + (x_t - prev)/TAU, prev = reset(m1_{t-1});
                # one instruction per group of gl timesteps (self-referencing).
                nc.vector._custom_dve(
                    lif_m1,
                    out=m1_buf[:, t0 + 1 : t0 + 1 + gl, :],
                    in0=x_slice(t0, gl),
                    in1=m1_buf[:, t0 : t0 + gl, :],
                    s0=0.5,      # 1/TAU
                    s1=1.0,      # V_TH
                )
                t0 += gl
                issue_oi_until(t0)
            assert oi_done == T and store_next is None

    nc.compile()
    _nc_cache[key] = nc
    return nc


def _shard(X: np.ndarray) -> list[np.ndarray]:
    """[B, T, N] -> per-core [(b j) = 128, T, F] contiguous."""
    Xt = np.ascontiguousarray(
        X.reshape(B, T, J, F).transpose(0, 2, 1, 3)
    )  # [B, J, T, F]
    return [
        Xt[c * BS : (c + 1) * BS].reshape(P, T, F) for c in range(CORES)
    ]


def _unshard(parts: list[np.ndarray]) -> np.ndarray:
    """per-core [(b j), T, F] -> [B, T, N]."""
    full = np.stack(parts).reshape(B, J, T, F)
    return np.ascontiguousarray(full.transpose(0, 2, 1, 3)).reshape(B, T, N)


def _run(X: np.ndarray, w: np.ndarray, **spmd_kwargs):
    X = np.asarray(X, dtype=np.float32)
    inv_tau = float(1.0 / (1.0 + np.exp(-np.float64(np.asarray(w).item()))))
    nc = _build(inv_tau)
    in_maps = [{"x": xs} for xs in _shard(X)]
    try:
        res = run_bass_kernel_spmd(
            nc, in_maps, core_ids=list(range(CORES)), **spmd_kwargs
        )
    except Exception:
        # One retry: device wedges (NRT_EXEC_UNIT_UNRECOVERABLE) are
        # occasionally transient; a clean re-run often succeeds.
        res = run_bass_kernel_spmd(
            nc, in_maps, core_ids=list(range(CORES)), **spmd_kwargs
        )
    i_pot = _unshard(
        [np.asarray(res.results[c]["oi"]).astype(np.float32) for c in range(CORES)]
    )
    # Exact spike recovery: oi_t = inv_tau * oi_{t-1} + spike_t, and the f16
    # rounding residual (<~1e-3) is far below the 0.5 rint threshold.
    prev = np.concatenate(
        [np.zeros((B, 1, N), np.float32), i_pot[:, :-1]], axis=1
    )
    spikes = np.rint(i_pot - np.float32(inv_tau) * prev).astype(np.float32)
    return (spikes, i_pot), res


def kernel(X: np.ndarray, w: np.ndarray):
    out, _ = _run(X, w)
    return out
